# revision 68
# baseline (speedup 1.0000x reference)
"""CapsuleRewardHead Trainium2 kernel (8-core data parallel), v2.

Math (per batch row b):
    primary = x @ W + b_lin                    [B, 128]  (128 = 8 caps x 16 dim)
    u_hat[b,o,i,j] = sum_c primary[b,i,c] * out_caps[o,i,c,j]
    3 rounds of dynamic routing over N=32 capsule pairs (o,i), D=16
    out[b] = |squash(s_final)|

Device strategy per core (2048 batch rows):
  - host: quantize x shard to fp8 e4m3, laid out [sp][128 part][hp][b] so each
    super is ONE contiguous 2MB DMA (16KB/partition) -> ~6us super latency,
    full 16-SDMA-engine spread, pipelined with MM1.
  - MM1 (PE): DoubleRow fp8 matmuls contract h-chunk pairs into PSUM:
    primaryT[ic, b] per 512-col super; linear bias rides as a K=1 bf16 matmul.
  - MM2 (PE): per 128-row chunk, TWO matmuls against differently-ordered
    block-diagonal caps constants give u_hat in both [K,N,D] (d-inner) and
    [K,D,N] (n-inner) layouts, plus a capsum matmul for round-0's t0.
  - routing: all elementwise on DVE with DIRECT broadcast reads (inner-step-1
    APs hit 2x mode on HW; verified by microbench — no erep/trep
    materialization, no GPSIMD which contends with DVE for the SBUF port).
    n-trees run on the n-inner copy, d-trees on the d-inner copy so every
    tree level is a 2x-mode halving add and the agreement lands directly in
    the [K,N] logit layout. sqrt via bit-trick seed; unnormalized
    accumulators (q = |t|^2, se = sum e).
  - emission order interleaves MM2 chunk blocks with group-0 rounds so ACT
    psum->sbuf copies never queue behind chain-critical exps.
"""

import os

import numpy as np
import ml_dtypes

B = 16384
HIDDEN = 4096
NUM_OBJ = 4
NUM_CAPS = 8
CAP_DIM = 16
N_ROUTE = 32  # NUM_OBJ * NUM_CAPS
N_CORES = 8

LAST_EXEC_TIME_NS = None  # set after each run when BASS_TRACE=1

BF16 = ml_dtypes.bfloat16
FP8 = ml_dtypes.float8_e4m3
W_SCALE = 1024.0
SQRT_MAGIC = 0x1FBD1DF5


def _ap(ap, dims):
    import concourse.bass as bass

    return bass.AP(tensor=ap.tensor, offset=ap.offset, ap=dims)


def build_bass(hidden=HIDDEN, b_sh=B // N_CORES, batch_plan=(4, 12)):
    import concourse.tile as tile
    from concourse import bacc, mybir

    NH = hidden // 128
    NCH = b_sh // 128  # 128-row chunks
    SUP = 512
    NSUP = b_sh // SUP
    CPS = SUP // 128
    assert sum(batch_plan) == NCH
    N, D = N_ROUTE, CAP_DIM
    dt = mybir.dt
    AX = mybir.AxisListType
    OP = mybir.AluOpType
    AF = mybir.ActivationFunctionType
    PM = mybir.MatmulPerfMode

    batches = []
    pos = 0
    for k in batch_plan:
        batches.append(list(range(pos, pos + k)))
        pos += k
    chunk_to_batch = {}
    for bi, chs in enumerate(batches):
        for ch in chs:
            chunk_to_batch[ch] = bi

    nc = bacc.Bacc("TRN2", target_bir_lowering=False, debug=False, num_devices=N_CORES)

    NPC = 4  # DMA pieces per super
    HQ = NH // NPC
    xt_ap = nc.dram_tensor(
        "xt", [NSUP, NPC, 128, HQ, SUP], dt.float8e4, kind="ExternalInput"
    ).ap()
    w_ap = nc.dram_tensor("w", [128, NH, 128], dt.float8e4, kind="ExternalInput").ap()
    capsd_ap = nc.dram_tensor(
        "capsd", [128, N * D], dt.bfloat16, kind="ExternalInput"
    ).ap()
    capsn_ap = nc.dram_tensor(
        "capsn", [128, D * N], dt.bfloat16, kind="ExternalInput"
    ).ap()
    capsum_ap = nc.dram_tensor(
        "capsum", [128, CAP_DIM], dt.bfloat16, kind="ExternalInput"
    ).ap()
    bias_ap = nc.dram_tensor("bias", [1, 256], dt.bfloat16, kind="ExternalInput").ap()
    ident_ap = nc.dram_tensor(
        "ident", [128, 128], dt.float32, kind="ExternalInput"
    ).ap()
    # [chunk, 128]: row-major flatten = batch order; 512B/partition descriptors
    out_ap = nc.dram_tensor("out", [NCH, 128], dt.float32, kind="ExternalOutput").ap()

    with tile.TileContext(nc) as tc:
        with (
            tc.tile_pool(name="singles", bufs=1) as singles,
            tc.tile_pool(name="xs", bufs=NSUP * NPC - 2) as xs_pool,
            tc.tile_pool(name="primt", bufs=3) as primt_pool,
            tc.tile_pool(name="batch", bufs=1) as bpool,
            tc.tile_pool(name="tmp", bufs=1) as tmp_pool,
            tc.tile_pool(name="sm", bufs=4) as sm_pool,
            tc.tile_pool(name="psum_p", bufs=2, space="PSUM") as psp_pool,
            tc.tile_pool(name="psum_u", bufs=3, space="PSUM") as psu_pool,
            tc.tile_pool(name="psum_t", bufs=1, space="PSUM") as pst_pool,
            tc.tile_pool(name="psum_w", bufs=1, space="PSUM") as psw_pool,
        ):
            w_sb = singles.tile([128, NH, 128], dt.float8e4)
            ident_sb = singles.tile([128, 128], dt.float32)
            outf_sb = singles.tile([16, NCH * 128 // 16], dt.float32)
            capsd_sb = singles.tile([128, N * D], dt.bfloat16)
            capsn_sb = singles.tile([128, D * N], dt.bfloat16)
            capsum_sb = singles.tile([128, CAP_DIM], dt.bfloat16)
            bias_sb = singles.tile([1, 256], dt.bfloat16)

            def issue_params():
                # qAct HWDGE ring so params don't delay the x stream on qSP;
                # smallest first so MM1's bias matmul unblocks earliest.
                # (w rides the qSP ring ahead of x - it gates every DR.)
                nc.scalar.dma_start(out=bias_sb[:], in_=bias_ap[:, :])
                nc.scalar.dma_start(out=capsum_sb[:], in_=capsum_ap[:, :])
                nc.scalar.dma_start(out=capsd_sb[:], in_=capsd_ap[:, :])
                nc.scalar.dma_start(out=capsn_sb[:], in_=capsn_ap[:, :])
                nc.scalar.dma_start(out=ident_sb[:], in_=ident_ap[:, :])

            magic_sb = singles.tile([128, 1], dt.uint32)
            nc.vector.memset(magic_sb[:], SQRT_MAGIC)
            out_sb = singles.tile([128, NCH], dt.float32)
            warm_sb = singles.tile([128, 2, SUP], dt.float8e4)
            nc.vector.memset(warm_sb.rearrange("p a b -> p (a b)"), 0)

            # D2 = D+1: a ones-plane rides along uh_nmaj so the n-tree
            # yields se = sum(e) for free in t[..., D], and den = q + se^2
            # falls out of one reduce over the squared 17-wide t.
            D2 = D + 1
            uhd_all, uhn_all, t_all, b_all = {}, {}, {}, {}
            for bi, chs in enumerate(batches):
                K = len(chs)
                uhd_all[bi] = bpool.tile(
                    [128, K, N, D], dt.bfloat16, tag=f"uhd{bi}", name=f"uhd{bi}"
                )
                uhn_all[bi] = bpool.tile(
                    [128, K, D2, N], dt.bfloat16, tag=f"uhn{bi}", name=f"uhn{bi}"
                )
                nc.vector.memset(uhn_all[bi][:, :, D, :], 1.0)
                t_all[bi] = bpool.tile(
                    [128, K, D2], dt.bfloat16, tag=f"t{bi}", name=f"t{bi}"
                )
                nc.vector.memset(t_all[bi][:, :, D], float(N))
                # two logit buffers: the r1 update writes out-of-place
                # (in-place DVE ops run ~4x slower), bf16 for 2x mode
                b_all[bi] = (
                    bpool.tile([128, K, N], dt.bfloat16, tag=f"b{bi}a",
                               name=f"b{bi}a"),
                    bpool.tile([128, K, N], dt.bfloat16, tag=f"b{bi}b",
                               name=f"b{bi}b"),
                )

            def smt(K, tag, dtype=dt.float32):
                return sm_pool.tile([128, K], dtype, tag=tag, name=tag)

            def sqrt_half(q, K):
                """bit-trick sqrt seed; error washes out through squash."""
                qu = q.bitcast(dt.uint32)
                s1 = smt(K, f"sq1_{K}", dt.uint32)
                nc.vector.tensor_single_scalar(
                    s1[:], qu, 1, op=OP.logical_shift_right
                )
                s2 = smt(K, f"sq2_{K}", dt.uint32)
                nc.vector.tensor_tensor(
                    s2[:],
                    s1[:],
                    _ap(magic_sb[:], [magic_sb[:].ap[0], [0, K]]),
                    op=OP.add,
                )
                return s2.bitcast(dt.float32)  # ~3.5% sqrt approx (validated)

            def tree_n(src, K, dst):
                """wm [128,K,D2,N] bf16 -> dst t [128,K,D2] via halving adds
                on innermost n (every level inner step 1 -> 2x mode).
                t[..., D] is se = sum(e) via the uh ones-plane."""
                cur = src
                w = N
                with nc.allow_low_precision(reason="tree bf16 validated"):
                    while w > 2:
                        w //= 2
                        nxt = tmp_pool.tile(
                            [128, K, D2, w], dt.bfloat16, tag=f"tn{w}",
                            name=f"tn{K}_{w}",
                        )
                        nc.vector.tensor_tensor(
                            nxt[:], cur[:, :, :, 0:w], cur[:, :, :, w : 2 * w],
                            op=OP.add,
                        )
                        cur = nxt
                    nc.vector.tensor_tensor(
                        dst, cur[:, :, :, 0], cur[:, :, :, 1], op=OP.add
                    )

            def qden(tt, K):
                """q = |t[0:D]|^2 and den = q + t[D]^2 from one squared tile
                (one reduce; q recovered as den - se^2)."""
                sqx = sm_pool.tile(
                    [128, K, D2], dt.bfloat16, tag=f"sqx{K}", name=f"sqx{K}"
                )
                nc.vector.tensor_tensor(sqx[:], tt[:], tt[:], op=OP.mult)
                den = smt(K, f"den{K}")
                nc.vector.tensor_reduce(den[:], sqx[:], axis=AX.X, op=OP.add)
                q = smt(K, f"q{K}")
                nc.vector.tensor_tensor(
                    q[:], den[:], sqx[:, :, D], op=OP.subtract
                )
                rden = smt(K, f"rden{K}")
                nc.vector.reciprocal(rden[:], den[:])
                return q, rden

            def tree_d(src, K, dst):
                """am [128,K,N,D] bf16 -> dst a [128,K,N] via halving adds on
                innermost d. dst lands directly in logit [K,N] layout.
                src is an AP (may be a sub-range of a wider tile)."""
                cur = src
                w = D
                with nc.allow_low_precision(reason="tree bf16 validated"):
                    while w > 2:
                        w //= 2
                        nxt = tmp_pool.tile(
                            [128, K, N, w], dt.bfloat16, tag=f"td{K}_{w}",
                            name=f"td{K}_{w}",
                        )
                        nc.vector.tensor_tensor(
                            nxt[:], cur[:, :, :, 0:w], cur[:, :, :, w : 2 * w],
                            op=OP.add,
                        )
                        cur = nxt
                    nc.vector.tensor_tensor(
                        dst, cur[:, :, :, 0], cur[:, :, :, 1], op=OP.add
                    )

            def t_bc(tt, K):
                t3 = tt[:, :, 0:D]
                return _ap(t3, [t3.ap[0], [D2, K], [0, N], [1, D]])

            am_r0, a0_all = {}, {}

            def routing_r0(bi, k0, k1, tree_now=False):
                """agreement multiply am = uh * t0_bc for chunks [k0,k1).
                tree_now: run the d-tree for this sub-range immediately
                (fills otherwise-idle DVE time during the stream head);
                else one merged tree runs in routing_r0_fin (fewer ops)."""
                K = len(batches[bi])
                uhd = uhd_all[bi]
                tt = t_all[bi]
                t3 = tt[:, k0:k1, 0:D]
                if bi not in am_r0:
                    am_r0[bi] = tmp_pool.tile(
                        [128, K, N, D], dt.bfloat16, tag=f"amg{bi}", name=f"amg{bi}"
                    )
                    a0_all[bi] = sm_pool.tile(
                        [128, K, N], dt.bfloat16, tag=f"a0{bi}", name=f"a0{bi}"
                    )
                am = am_r0[bi]
                nc.vector.tensor_tensor(
                    am[:, k0:k1, :, :].rearrange("p a b c -> p (a b c)"),
                    uhd[:, k0:k1, :, :].rearrange("p a b c -> p (a b c)"),
                    _ap(t3, [t3.ap[0], [D2, k1 - k0], [0, N], [1, D]]),
                    op=OP.mult,
                )
                if tree_now:
                    tree_d(
                        am[:, k0:k1, :, :], k1 - k0, a0_all[bi][:, k0:k1, :]
                    )
                    am_r0[bi] = am_r0[bi]  # keep tile alive for other halves

            def routing_r0_fin(bi, tree_done=False):
                """(remaining) d-tree + scalar chain + b1 = alpha0 * a0."""
                chs = batches[bi]
                K = len(chs)
                tt = t_all[bi]
                bA, _ = b_all[bi]
                a0 = a0_all.pop(bi)
                if not tree_done:
                    tree_d(am_r0[bi][:], K, a0[:])
                am_r0.pop(bi)
                q, rden = qden(tt, K)
                sm = sqrt_half(q[:], K)
                alpha = smt(K, f"alpha{K}")
                nc.vector.tensor_mul(alpha[:], sm, rden[:])
                # b1 = alpha_bc * a0  (alpha broadcast along n: inner step 0
                # -> 1x, but FD is only K*N)
                nc.vector.tensor_tensor(
                    bA[:],
                    a0[:],
                    _ap(alpha[:], [*alpha[:].ap, [0, N]]),
                    op=OP.mult,
                )

            def mm2_d_vec(sp):
                """like mm2_d but uhd copies on the (idle) Vector engine -
                used for super 0 so the head chain isn't ACT-serialized."""
                for c in range(CPS):
                    s = sp * CPS + c
                    bi = chunk_to_batch[s]
                    k = s - batches[bi][0]
                    lhsT = primt_all[sp][:, c * 128 : (c + 1) * 128]
                    psu_d = psu_pool.tile(
                        [128, N * D], dt.float32, tag="psu", name="psu_d"
                    )
                    nc.tensor.matmul(
                        psu_d[:], lhsT, capsd_sb[:], start=True, stop=True
                    )
                    pst = pst_pool.tile([128, CAP_DIM], dt.float32)
                    nc.tensor.matmul(
                        pst[:], lhsT, capsum_sb[:], start=True, stop=True
                    )
                    nc.vector.tensor_copy(
                        uhd_all[bi][:, k, :, :],
                        psu_d.rearrange("p (n d) -> p n d", n=N),
                    )
                    nc.scalar.copy(t_all[bi][:, k, 0:D], pst[:])

            def routing_round(bi, r):
                """rounds 1..2: softmax-weighted sum + (r==1) agreement."""
                chs = batches[bi]
                K = len(chs)
                uhd, uhn = uhd_all[bi], uhn_all[bi]
                tt = t_all[bi]
                bA, bB = b_all[bi]
                bcur = bA if r == 1 else bB
                if r == 2:
                    # r2 logits can reach ~56; subtract the max so se^2
                    # stays in fp32 range. r1 logits are <~33, exp directly.
                    mx = smt(K, f"mx{K}", dt.bfloat16)
                    with nc.allow_low_precision(reason="bf16 logits"):
                        nc.vector.tensor_reduce(
                            mx[:], bcur[:], axis=AX.X, op=OP.max
                        )
                    bsub = sm_pool.tile(
                        [128, K, N], dt.bfloat16, tag=f"bsub{K}", name=f"bsub{K}"
                    )
                    nc.vector.tensor_tensor(
                        bsub[:],
                        bcur[:],
                        _ap(mx[:], [*mx[:].ap, [0, N]]),
                        op=OP.subtract,
                    )
                    esrc = bsub[:]
                else:
                    esrc = bcur[:]
                e = sm_pool.tile([128, K, N], dt.bfloat16, tag=f"esm{K}", name=f"esm{K}")
                nc.scalar.activation(e[:], esrc, AF.Exp)
                # wm = uh_nmaj * e  (e broadcast along d2: [0,D2] outer,
                # [1,N] inner -> 2x mode, no materialization); the ones-plane
                # row makes the tree emit se = sum(e) in t[..., D].
                e3 = e[:]
                wm = tmp_pool.tile(
                    [128, K, D2, N], dt.bfloat16, tag=f"wm{K}", name=f"wm{K}"
                )
                nc.vector.tensor_tensor(
                    wm.rearrange("p a b c -> p (a b c)"),
                    uhn[:].rearrange("p a b c -> p (a b c)"),
                    _ap(e3, [e3.ap[0], [N, K], [0, D2], [1, N]]),
                    op=OP.mult,
                )
                tree_n(wm, K, tt[:])
                q, rden = qden(tt, K)
                if r == 1:
                    sm = sqrt_half(q[:], K)
                    alpha = smt(K, f"alpha{K}")
                    nc.vector.tensor_mul(alpha[:], sm, rden[:])
                    am = tmp_pool.tile(
                        [128, K, N, D], dt.bfloat16, tag=f"am{K}", name=f"am{K}"
                    )
                    nc.vector.tensor_tensor(
                        am.rearrange("p a b c -> p (a b c)"),
                        uhd[:].rearrange("p a b c -> p (a b c)"),
                        t_bc(tt, K),
                        op=OP.mult,
                    )
                    a1 = sm_pool.tile(
                        [128, K, N], dt.bfloat16, tag=f"a1_{K}", name=f"a1_{K}"
                    )
                    tree_d(am, K, a1[:])
                    badd = sm_pool.tile(
                        [128, K, N], dt.bfloat16, tag=f"badd{K}", name=f"badd{K}"
                    )
                    nc.vector.tensor_tensor(
                        badd[:],
                        a1[:],
                        _ap(alpha[:], [*alpha[:].ap, [0, N]]),
                        op=OP.mult,
                    )
                    with nc.allow_low_precision(reason="bf16 logits"):
                        nc.vector.tensor_tensor(
                            bB[:], bA[:], badd[:], op=OP.add
                        )
                else:
                    nc.vector.tensor_mul(
                        out_sb[:, chs[0] : chs[0] + K], q[:], rden[:]
                    )

            # ---- w first on the qSP ring (gates every DR matmul), then the
            # 16 x piece-DMAs pipeline behind it; small params on qAct.
            nc.sync.dma_start(out=w_sb[:], in_=w_ap[:, :, :])
            issue_params()
            xs_tiles = {}
            for sp in range(NSUP):
                for pc in range(NPC):
                    xs = xs_pool.tile([128, HQ, SUP], dt.float8e4)
                    nc.sync.dma_start(out=xs[:], in_=xt_ap[sp, pc])
                    xs_tiles[(sp, pc)] = xs

            # PE p-state warmup: >=3.4us sustained so HAM reaches 8/8
            # right as the first piece lands
            psw = psw_pool.tile([128, SUP], dt.float32)
            for wi in range(11):
                nc.tensor.matmul(
                    psw[:],
                    warm_sb[:, 0, 0:128],
                    warm_sb[:, 1, :],
                    start=(wi == 0),
                    stop=(wi == 10),
                )

            primt_all = {}

            def mm1_super(sp):
                psp = psp_pool.tile([128, SUP], dt.float32)
                # Linear bias rides as a K=1 bf16 matmul against ones
                ones_bc = _ap(
                    bias_sb[:, 128:256],
                    [bias_sb[:, 128:256].ap[0], [0, CPS], [1, 128]],
                )
                nc.tensor.matmul(
                    psp[:], bias_sb[:, 0:128], ones_bc, start=True, stop=False
                )
                for pc in range(NPC):
                    xs = xs_tiles[(sp, pc)]
                    for hp in range(HQ // 2):
                        h = pc * HQ + 2 * hp
                        nc.tensor.matmul(
                            psp[:],
                            w_sb[:, h : h + 2, :],
                            xs[:, 2 * hp : 2 * hp + 2, :],
                            start=False,
                            stop=(pc == NPC - 1 and hp == HQ // 2 - 1),
                            perf_mode=PM.DoubleRow,
                        )
                primt = primt_pool.tile([128, SUP], dt.bfloat16)
                nc.scalar.copy(primt[:], psp[:])
                primt_all[sp] = primt

            def mm2_d(sp, split=False):
                """uh_dmaj + t0 for the 4 chunks of super sp (feeds r0).
                split: do half the psum->sbuf copies on the (idle) Vector
                engine so the ACT copy chain isn't serial."""
                for c in range(CPS):
                    s = sp * CPS + c
                    bi = chunk_to_batch[s]
                    k = s - batches[bi][0]
                    lhsT = primt_all[sp][:, c * 128 : (c + 1) * 128]
                    psu_d = psu_pool.tile(
                        [128, N * D], dt.float32, tag="psu", name="psu_d"
                    )
                    nc.tensor.matmul(
                        psu_d[:], lhsT, capsd_sb[:], start=True, stop=True
                    )
                    pst = pst_pool.tile([128, CAP_DIM], dt.float32)
                    nc.tensor.matmul(
                        pst[:], lhsT, capsum_sb[:], start=True, stop=True
                    )
                    dst = uhd_all[bi][:, k, :, :]
                    src = psu_d.rearrange("p (n d) -> p n d", n=N)
                    if split and c >= 2:
                        nc.vector.tensor_copy(dst, src)
                    else:
                        nc.scalar.copy(dst, src)
                    nc.scalar.copy(t_all[bi][:, k, 0:D], pst[:])

            def mm2_n(sp, split=False):
                """uh_nmaj for the 4 chunks of super sp (feeds r1/r2)."""
                for c in range(CPS):
                    s = sp * CPS + c
                    bi = chunk_to_batch[s]
                    k = s - batches[bi][0]
                    lhsT = primt_all[sp][:, c * 128 : (c + 1) * 128]
                    psu_n = psu_pool.tile(
                        [128, D * N], dt.float32, tag="psu", name="psu_n"
                    )
                    nc.tensor.matmul(
                        psu_n[:], lhsT, capsn_sb[:], start=True, stop=True
                    )
                    dst = uhn_all[bi][:, k, 0:D, :]
                    src = psu_n.rearrange("p (d n) -> p d n", d=D)
                    if split and c >= 2:
                        nc.vector.tensor_copy(dst, src)
                    else:
                        nc.scalar.copy(dst, src)

            # ---- emission schedule: two groups of 8 chunks (2 supers each),
            # ordered by data arrival so no engine FIFO entry blocks a
            # later-emitted but earlier-ready op.
            mm1_super(0)
            mm2_d(0, split=True)
            routing_r0(0, 0, 4, tree_now=True)
            mm2_n(0, split=True)
            routing_r0_fin(0, tree_done=True)
            routing_round(0, 1)
            mm1_super(1)
            mm2_d(1, split=True)
            routing_r0(1, 0, 4, tree_now=True)
            mm2_n(1)
            routing_round(0, 2)
            mm1_super(2)
            mm2_d(2)
            routing_r0(1, 4, 8, tree_now=True)
            mm2_n(2)
            mm1_super(3)
            mm2_d(3)
            routing_r0(1, 8, 12, tree_now=True)
            routing_r0_fin(1, tree_done=True)
            mm2_n(3)
            routing_round(1, 1)
            routing_round(1, 2)
            # transpose [128 rows, 16 chunks] -> [16, 128] on PE so the
            # output leaves as ONE dma with 512B/partition descriptors
            # (the naive scattered store cost ~11us of tail)
            ot = pst_pool.tile([16, 128], dt.float32, tag="otr", name="otr")
            nc.tensor.transpose(ot[:], out_sb[:], ident_sb[:])
            nc.scalar.copy(outf_sb[:], ot[:])
            nc.sync.dma_start(out=out_ap[:, :], in_=outf_sb[:])

    nc.compile()
    return nc


def _prep_params(W, b_lin, out_caps, hidden=HIDDEN):
    NH = hidden // 128
    w_f = np.ascontiguousarray(
        (W.astype(np.float32) * W_SCALE)
        .reshape(NH, 128, NUM_CAPS * CAP_DIM)
        .transpose(1, 0, 2)
    ).astype(FP8)
    # full_caps[ic, n, d]: block-diagonal per (o,i): rows i*16..i*16+15
    full_caps = np.zeros((128, N_ROUTE, CAP_DIM), np.float32)
    for o in range(NUM_OBJ):
        for i in range(NUM_CAPS):
            full_caps[
                i * CAP_DIM : (i + 1) * CAP_DIM, o * NUM_CAPS + i, :
            ] = out_caps[o, i]
    full_caps /= W_SCALE
    capsd = np.ascontiguousarray(full_caps.reshape(128, -1)).astype(BF16)
    capsn = np.ascontiguousarray(
        full_caps.transpose(0, 2, 1).reshape(128, -1)
    ).astype(BF16)
    capsum = np.ascontiguousarray(full_caps.sum(1)).astype(BF16)
    bias_row = np.concatenate(
        [
            b_lin.astype(np.float32).reshape(1, 128) * W_SCALE,
            np.ones((1, 128), np.float32),
        ],
        axis=1,
    ).astype(BF16)
    return w_f, capsd, capsn, capsum, bias_row


_NC_CACHE = {}


def kernel(x, W, b_lin, out_caps):
    global LAST_EXEC_TIME_NS
    from concourse.bass_utils import run_bass_kernel_spmd

    x = np.asarray(x)
    W = np.asarray(W)
    b_lin = np.asarray(b_lin)
    out_caps = np.asarray(out_caps)
    bsz, hidden = x.shape
    b_sh = bsz // N_CORES
    NH = hidden // 128
    SUP = 512
    NSUP = b_sh // SUP

    key = (hidden, b_sh)
    if key not in _NC_CACHE:
        _NC_CACHE[key] = build_bass(hidden=hidden, b_sh=b_sh)
    nc = _NC_CACHE[key]

    w_f, capsd, capsn, capsum, bias_row = _prep_params(W, b_lin, out_caps, hidden)

    in_maps = []
    for i in range(N_CORES):
        shard = x[i * b_sh : (i + 1) * b_sh]
        # [sp, pc, p, hq, b]: 4 contiguous 512KB piece-DMAs per super
        # (4KB contiguous per partition per piece)
        NPC = 4
        HQ = NH // NPC
        xt = np.ascontiguousarray(
            shard.reshape(NSUP, SUP, NPC, HQ, 128).transpose(0, 2, 4, 3, 1)
        ).astype(FP8)
        in_maps.append(
            {
                "xt": xt,
                "w": w_f,
                "capsd": capsd,
                "capsn": capsn,
                "capsum": capsum,
                "bias": bias_row,
                "ident": np.eye(128, dtype=np.float32),
            }
        )

    res = run_bass_kernel_spmd(
        nc,
        in_maps,
        core_ids=list(range(N_CORES)),
        trace=bool(int(os.environ.get("BASS_TRACE", "0") or "0")),
    )
    LAST_EXEC_TIME_NS = res.exec_time_ns
    return np.concatenate(
        [res.results[i]["out"].reshape(-1) for i in range(N_CORES)]
    )


# revision 69
# speedup vs baseline: 1.0091x; 1.0091x over previous
"""CapsuleRewardHead Trainium2 kernel (8-core data parallel), v2.

Math (per batch row b):
    primary = x @ W + b_lin                    [B, 128]  (128 = 8 caps x 16 dim)
    u_hat[b,o,i,j] = sum_c primary[b,i,c] * out_caps[o,i,c,j]
    3 rounds of dynamic routing over N=32 capsule pairs (o,i), D=16
    out[b] = |squash(s_final)|

Device strategy per core (2048 batch rows):
  - host: quantize x shard to fp8 e4m3, laid out [sp][128 part][hp][b] so each
    super is ONE contiguous 2MB DMA (16KB/partition) -> ~6us super latency,
    full 16-SDMA-engine spread, pipelined with MM1.
  - MM1 (PE): DoubleRow fp8 matmuls contract h-chunk pairs into PSUM:
    primaryT[ic, b] per 512-col super; linear bias rides as a K=1 bf16 matmul.
  - MM2 (PE): per 128-row chunk, TWO matmuls against differently-ordered
    block-diagonal caps constants give u_hat in both [K,N,D] (d-inner) and
    [K,D,N] (n-inner) layouts, plus a capsum matmul for round-0's t0.
  - routing: all elementwise on DVE with DIRECT broadcast reads (inner-step-1
    APs hit 2x mode on HW; verified by microbench — no erep/trep
    materialization, no GPSIMD which contends with DVE for the SBUF port).
    n-trees run on the n-inner copy, d-trees on the d-inner copy so every
    tree level is a 2x-mode halving add and the agreement lands directly in
    the [K,N] logit layout. sqrt via bit-trick seed; unnormalized
    accumulators (q = |t|^2, se = sum e).
  - emission order interleaves MM2 chunk blocks with group-0 rounds so ACT
    psum->sbuf copies never queue behind chain-critical exps.
"""

import os

import numpy as np
import ml_dtypes

B = 16384
HIDDEN = 4096
NUM_OBJ = 4
NUM_CAPS = 8
CAP_DIM = 16
N_ROUTE = 32  # NUM_OBJ * NUM_CAPS
N_CORES = 8

LAST_EXEC_TIME_NS = None  # set after each run when BASS_TRACE=1

BF16 = ml_dtypes.bfloat16
FP8 = ml_dtypes.float8_e4m3
W_SCALE = 1024.0
SQRT_MAGIC = 0x1FBD1DF5


def _ap(ap, dims):
    import concourse.bass as bass

    return bass.AP(tensor=ap.tensor, offset=ap.offset, ap=dims)


def build_bass(hidden=HIDDEN, b_sh=B // N_CORES, batch_plan=(6, 10)):
    import concourse.tile as tile
    from concourse import bacc, mybir

    NH = hidden // 128
    NCH = b_sh // 128  # 128-row chunks
    SUP = 512
    NSUP = b_sh // SUP
    CPS = SUP // 128
    assert sum(batch_plan) == NCH
    N, D = N_ROUTE, CAP_DIM
    dt = mybir.dt
    AX = mybir.AxisListType
    OP = mybir.AluOpType
    AF = mybir.ActivationFunctionType
    PM = mybir.MatmulPerfMode

    batches = []
    pos = 0
    for k in batch_plan:
        batches.append(list(range(pos, pos + k)))
        pos += k
    chunk_to_batch = {}
    for bi, chs in enumerate(batches):
        for ch in chs:
            chunk_to_batch[ch] = bi

    nc = bacc.Bacc("TRN2", target_bir_lowering=False, debug=False, num_devices=N_CORES)

    NPC = 4  # DMA pieces per super
    HQ = NH // NPC
    xt_ap = nc.dram_tensor(
        "xt", [NSUP, NPC, 128, HQ, SUP], dt.float8e4, kind="ExternalInput"
    ).ap()
    w_ap = nc.dram_tensor("w", [128, NH, 128], dt.float8e4, kind="ExternalInput").ap()
    capsd_ap = nc.dram_tensor(
        "capsd", [128, N * D], dt.bfloat16, kind="ExternalInput"
    ).ap()
    capsn_ap = nc.dram_tensor(
        "capsn", [128, D * N], dt.bfloat16, kind="ExternalInput"
    ).ap()
    capsum_ap = nc.dram_tensor(
        "capsum", [128, CAP_DIM], dt.bfloat16, kind="ExternalInput"
    ).ap()
    bias_ap = nc.dram_tensor("bias", [1, 256], dt.bfloat16, kind="ExternalInput").ap()
    ident_ap = nc.dram_tensor(
        "ident", [128, 128], dt.float32, kind="ExternalInput"
    ).ap()
    # [chunk, 128]: row-major flatten = batch order; 512B/partition descriptors
    out_ap = nc.dram_tensor("out", [NCH, 128], dt.float32, kind="ExternalOutput").ap()

    with tile.TileContext(nc) as tc:
        with (
            tc.tile_pool(name="singles", bufs=1) as singles,
            tc.tile_pool(name="xs", bufs=NSUP * NPC - 2) as xs_pool,
            tc.tile_pool(name="primt", bufs=3) as primt_pool,
            tc.tile_pool(name="batch", bufs=1) as bpool,
            tc.tile_pool(name="tmp", bufs=1) as tmp_pool,
            tc.tile_pool(name="sm", bufs=4) as sm_pool,
            tc.tile_pool(name="psum_p", bufs=2, space="PSUM") as psp_pool,
            tc.tile_pool(name="psum_u", bufs=3, space="PSUM") as psu_pool,
            tc.tile_pool(name="psum_t", bufs=1, space="PSUM") as pst_pool,
            tc.tile_pool(name="psum_w", bufs=1, space="PSUM") as psw_pool,
        ):
            w_sb = singles.tile([128, NH, 128], dt.float8e4)
            ident_sb = singles.tile([128, 128], dt.float32)
            outf_sb = singles.tile([16, NCH * 128 // 16], dt.float32)
            capsd_sb = singles.tile([128, N * D], dt.bfloat16)
            capsn_sb = singles.tile([128, D * N], dt.bfloat16)
            capsum_sb = singles.tile([128, CAP_DIM], dt.bfloat16)
            bias_sb = singles.tile([1, 256], dt.bfloat16)

            def issue_params():
                # qAct HWDGE ring so params don't delay the x stream on qSP;
                # smallest first so MM1's bias matmul unblocks earliest.
                # (w rides the qSP ring ahead of x - it gates every DR.)
                nc.scalar.dma_start(out=bias_sb[:], in_=bias_ap[:, :])
                nc.scalar.dma_start(out=capsum_sb[:], in_=capsum_ap[:, :])
                nc.scalar.dma_start(out=capsd_sb[:], in_=capsd_ap[:, :])
                nc.scalar.dma_start(out=capsn_sb[:], in_=capsn_ap[:, :])
                nc.scalar.dma_start(out=ident_sb[:], in_=ident_ap[:, :])

            magic_sb = singles.tile([128, 1], dt.uint32)
            nc.vector.memset(magic_sb[:], SQRT_MAGIC)
            out_sb = singles.tile([128, NCH], dt.float32)
            warm_sb = singles.tile([128, 2, SUP], dt.float8e4)
            nc.vector.memset(warm_sb.rearrange("p a b -> p (a b)"), 0)

            # D2 = D+1: a ones-plane rides along uh_nmaj so the n-tree
            # yields se = sum(e) for free in t[..., D], and den = q + se^2
            # falls out of one reduce over the squared 17-wide t.
            D2 = D + 1
            uhd_all, uhn_all, t_all, b_all = {}, {}, {}, {}
            for bi, chs in enumerate(batches):
                K = len(chs)
                uhd_all[bi] = bpool.tile(
                    [128, K, N, D], dt.bfloat16, tag=f"uhd{bi}", name=f"uhd{bi}"
                )
                uhn_all[bi] = bpool.tile(
                    [128, K, D2, N], dt.bfloat16, tag=f"uhn{bi}", name=f"uhn{bi}"
                )
                nc.vector.memset(uhn_all[bi][:, :, D, :], 1.0)
                t_all[bi] = bpool.tile(
                    [128, K, D2], dt.bfloat16, tag=f"t{bi}", name=f"t{bi}"
                )
                nc.vector.memset(t_all[bi][:, :, D], float(N))
                # two logit buffers: the r1 update writes out-of-place
                # (in-place DVE ops run ~4x slower), bf16 for 2x mode
                b_all[bi] = (
                    bpool.tile([128, K, N], dt.bfloat16, tag=f"b{bi}a",
                               name=f"b{bi}a"),
                    bpool.tile([128, K, N], dt.bfloat16, tag=f"b{bi}b",
                               name=f"b{bi}b"),
                )

            def smt(K, tag, dtype=dt.float32):
                return sm_pool.tile([128, K], dtype, tag=tag, name=tag)

            def sqrt_half(q, K):
                """bit-trick sqrt seed; error washes out through squash."""
                qu = q.bitcast(dt.uint32)
                s1 = smt(K, f"sq1_{K}", dt.uint32)
                nc.vector.tensor_single_scalar(
                    s1[:], qu, 1, op=OP.logical_shift_right
                )
                s2 = smt(K, f"sq2_{K}", dt.uint32)
                nc.vector.tensor_tensor(
                    s2[:],
                    s1[:],
                    _ap(magic_sb[:], [magic_sb[:].ap[0], [0, K]]),
                    op=OP.add,
                )
                return s2.bitcast(dt.float32)  # ~3.5% sqrt approx (validated)

            def tree_n(src, K, dst):
                """wm [128,K,D2,N] bf16 -> dst t [128,K,D2] via halving adds
                on innermost n (every level inner step 1 -> 2x mode).
                t[..., D] is se = sum(e) via the uh ones-plane."""
                cur = src
                w = N
                with nc.allow_low_precision(reason="tree bf16 validated"):
                    while w > 2:
                        w //= 2
                        nxt = tmp_pool.tile(
                            [128, K, D2, w], dt.bfloat16, tag=f"tn{w}",
                            name=f"tn{K}_{w}",
                        )
                        nc.vector.tensor_tensor(
                            nxt[:], cur[:, :, :, 0:w], cur[:, :, :, w : 2 * w],
                            op=OP.add,
                        )
                        cur = nxt
                    nc.vector.tensor_tensor(
                        dst, cur[:, :, :, 0], cur[:, :, :, 1], op=OP.add
                    )

            def qden(tt, K):
                """q = |t[0:D]|^2 and den = q + t[D]^2 from one squared tile
                (one reduce; q recovered as den - se^2)."""
                sqx = sm_pool.tile(
                    [128, K, D2], dt.bfloat16, tag=f"sqx{K}", name=f"sqx{K}"
                )
                nc.vector.tensor_tensor(sqx[:], tt[:], tt[:], op=OP.mult)
                den = smt(K, f"den{K}")
                nc.vector.tensor_reduce(den[:], sqx[:], axis=AX.X, op=OP.add)
                q = smt(K, f"q{K}")
                nc.vector.tensor_tensor(
                    q[:], den[:], sqx[:, :, D], op=OP.subtract
                )
                rden = smt(K, f"rden{K}")
                nc.vector.reciprocal(rden[:], den[:])
                return q, rden

            def tree_d(src, K, dst):
                """am [128,K,N,D] bf16 -> dst a [128,K,N] via halving adds on
                innermost d. dst lands directly in logit [K,N] layout.
                src is an AP (may be a sub-range of a wider tile)."""
                cur = src
                w = D
                with nc.allow_low_precision(reason="tree bf16 validated"):
                    while w > 2:
                        w //= 2
                        nxt = tmp_pool.tile(
                            [128, K, N, w], dt.bfloat16, tag=f"td{K}_{w}",
                            name=f"td{K}_{w}",
                        )
                        nc.vector.tensor_tensor(
                            nxt[:], cur[:, :, :, 0:w], cur[:, :, :, w : 2 * w],
                            op=OP.add,
                        )
                        cur = nxt
                    nc.vector.tensor_tensor(
                        dst, cur[:, :, :, 0], cur[:, :, :, 1], op=OP.add
                    )

            def t_bc(tt, K):
                t3 = tt[:, :, 0:D]
                return _ap(t3, [t3.ap[0], [D2, K], [0, N], [1, D]])

            am_r0, a0_all = {}, {}

            def routing_r0(bi, k0, k1, tree_now=False):
                """agreement multiply am = uh * t0_bc for chunks [k0,k1).
                tree_now: run the d-tree for this sub-range immediately
                (fills otherwise-idle DVE time during the stream head);
                else one merged tree runs in routing_r0_fin (fewer ops)."""
                K = len(batches[bi])
                uhd = uhd_all[bi]
                tt = t_all[bi]
                t3 = tt[:, k0:k1, 0:D]
                if bi not in am_r0:
                    am_r0[bi] = tmp_pool.tile(
                        [128, K, N, D], dt.bfloat16, tag=f"amg{bi}", name=f"amg{bi}"
                    )
                    a0_all[bi] = sm_pool.tile(
                        [128, K, N], dt.bfloat16, tag=f"a0{bi}", name=f"a0{bi}"
                    )
                am = am_r0[bi]
                nc.vector.tensor_tensor(
                    am[:, k0:k1, :, :].rearrange("p a b c -> p (a b c)"),
                    uhd[:, k0:k1, :, :].rearrange("p a b c -> p (a b c)"),
                    _ap(t3, [t3.ap[0], [D2, k1 - k0], [0, N], [1, D]]),
                    op=OP.mult,
                )
                if tree_now:
                    tree_d(
                        am[:, k0:k1, :, :], k1 - k0, a0_all[bi][:, k0:k1, :]
                    )
                    am_r0[bi] = am_r0[bi]  # keep tile alive for other halves

            def routing_r0_fin(bi, tree_done=False):
                """(remaining) d-tree + scalar chain + b1 = alpha0 * a0."""
                chs = batches[bi]
                K = len(chs)
                tt = t_all[bi]
                bA, _ = b_all[bi]
                a0 = a0_all.pop(bi)
                if not tree_done:
                    tree_d(am_r0[bi][:], K, a0[:])
                am_r0.pop(bi)
                q, rden = qden(tt, K)
                sm = sqrt_half(q[:], K)
                alpha = smt(K, f"alpha{K}")
                nc.vector.tensor_mul(alpha[:], sm, rden[:])
                # b1 = alpha_bc * a0  (alpha broadcast along n: inner step 0
                # -> 1x, but FD is only K*N)
                nc.vector.tensor_tensor(
                    bA[:],
                    a0[:],
                    _ap(alpha[:], [*alpha[:].ap, [0, N]]),
                    op=OP.mult,
                )

            def mm2_d_vec(sp):
                """like mm2_d but uhd copies on the (idle) Vector engine -
                used for super 0 so the head chain isn't ACT-serialized."""
                for c in range(CPS):
                    s = sp * CPS + c
                    bi = chunk_to_batch[s]
                    k = s - batches[bi][0]
                    lhsT = primt_all[sp][:, c * 128 : (c + 1) * 128]
                    psu_d = psu_pool.tile(
                        [128, N * D], dt.float32, tag="psu", name="psu_d"
                    )
                    nc.tensor.matmul(
                        psu_d[:], lhsT, capsd_sb[:], start=True, stop=True
                    )
                    pst = pst_pool.tile([128, CAP_DIM], dt.float32)
                    nc.tensor.matmul(
                        pst[:], lhsT, capsum_sb[:], start=True, stop=True
                    )
                    nc.vector.tensor_copy(
                        uhd_all[bi][:, k, :, :],
                        psu_d.rearrange("p (n d) -> p n d", n=N),
                    )
                    nc.scalar.copy(t_all[bi][:, k, 0:D], pst[:])

            def routing_round(bi, r):
                """rounds 1..2: softmax-weighted sum + (r==1) agreement."""
                chs = batches[bi]
                K = len(chs)
                uhd, uhn = uhd_all[bi], uhn_all[bi]
                tt = t_all[bi]
                bA, bB = b_all[bi]
                bcur = bA if r == 1 else bB
                if r == 2:
                    # r2 logits can reach ~56; subtract the max so se^2
                    # stays in fp32 range. r1 logits are <~33, exp directly.
                    mx = smt(K, f"mx{K}", dt.bfloat16)
                    with nc.allow_low_precision(reason="bf16 logits"):
                        nc.vector.tensor_reduce(
                            mx[:], bcur[:], axis=AX.X, op=OP.max
                        )
                    bsub = sm_pool.tile(
                        [128, K, N], dt.bfloat16, tag=f"bsub{K}", name=f"bsub{K}"
                    )
                    nc.vector.tensor_tensor(
                        bsub[:],
                        bcur[:],
                        _ap(mx[:], [*mx[:].ap, [0, N]]),
                        op=OP.subtract,
                    )
                    esrc = bsub[:]
                else:
                    esrc = bcur[:]
                e = sm_pool.tile([128, K, N], dt.bfloat16, tag=f"esm{K}", name=f"esm{K}")
                nc.scalar.activation(e[:], esrc, AF.Exp)
                # wm = uh_nmaj * e  (e broadcast along d2: [0,D2] outer,
                # [1,N] inner -> 2x mode, no materialization); the ones-plane
                # row makes the tree emit se = sum(e) in t[..., D].
                e3 = e[:]
                wm = tmp_pool.tile(
                    [128, K, D2, N], dt.bfloat16, tag=f"wm{K}", name=f"wm{K}"
                )
                nc.vector.tensor_tensor(
                    wm.rearrange("p a b c -> p (a b c)"),
                    uhn[:].rearrange("p a b c -> p (a b c)"),
                    _ap(e3, [e3.ap[0], [N, K], [0, D2], [1, N]]),
                    op=OP.mult,
                )
                tree_n(wm, K, tt[:])
                q, rden = qden(tt, K)
                if r == 1:
                    sm = sqrt_half(q[:], K)
                    alpha = smt(K, f"alpha{K}")
                    nc.vector.tensor_mul(alpha[:], sm, rden[:])
                    am = tmp_pool.tile(
                        [128, K, N, D], dt.bfloat16, tag=f"am{K}", name=f"am{K}"
                    )
                    nc.vector.tensor_tensor(
                        am.rearrange("p a b c -> p (a b c)"),
                        uhd[:].rearrange("p a b c -> p (a b c)"),
                        t_bc(tt, K),
                        op=OP.mult,
                    )
                    a1 = sm_pool.tile(
                        [128, K, N], dt.bfloat16, tag=f"a1_{K}", name=f"a1_{K}"
                    )
                    tree_d(am, K, a1[:])
                    badd = sm_pool.tile(
                        [128, K, N], dt.bfloat16, tag=f"badd{K}", name=f"badd{K}"
                    )
                    nc.vector.tensor_tensor(
                        badd[:],
                        a1[:],
                        _ap(alpha[:], [*alpha[:].ap, [0, N]]),
                        op=OP.mult,
                    )
                    with nc.allow_low_precision(reason="bf16 logits"):
                        nc.vector.tensor_tensor(
                            bB[:], bA[:], badd[:], op=OP.add
                        )
                else:
                    nc.vector.tensor_mul(
                        out_sb[:, chs[0] : chs[0] + K], q[:], rden[:]
                    )

            # ---- w first on the qSP ring (gates every DR matmul), then the
            # 16 x piece-DMAs pipeline behind it; small params on qAct.
            nc.sync.dma_start(out=w_sb[:], in_=w_ap[:, :, :])
            issue_params()
            xs_tiles = {}
            for sp in range(NSUP):
                for pc in range(NPC):
                    xs = xs_pool.tile([128, HQ, SUP], dt.float8e4)
                    nc.sync.dma_start(out=xs[:], in_=xt_ap[sp, pc])
                    xs_tiles[(sp, pc)] = xs

            # PE p-state warmup: >=3.4us sustained so HAM reaches 8/8
            # right as the first piece lands
            psw = psw_pool.tile([128, SUP], dt.float32)
            for wi in range(11):
                nc.tensor.matmul(
                    psw[:],
                    warm_sb[:, 0, 0:128],
                    warm_sb[:, 1, :],
                    start=(wi == 0),
                    stop=(wi == 10),
                )

            primt_all = {}

            def mm1_super(sp):
                psp = psp_pool.tile([128, SUP], dt.float32)
                # Linear bias rides as a K=1 bf16 matmul against ones
                ones_bc = _ap(
                    bias_sb[:, 128:256],
                    [bias_sb[:, 128:256].ap[0], [0, CPS], [1, 128]],
                )
                nc.tensor.matmul(
                    psp[:], bias_sb[:, 0:128], ones_bc, start=True, stop=False
                )
                for pc in range(NPC):
                    xs = xs_tiles[(sp, pc)]
                    for hp in range(HQ // 2):
                        h = pc * HQ + 2 * hp
                        nc.tensor.matmul(
                            psp[:],
                            w_sb[:, h : h + 2, :],
                            xs[:, 2 * hp : 2 * hp + 2, :],
                            start=False,
                            stop=(pc == NPC - 1 and hp == HQ // 2 - 1),
                            perf_mode=PM.DoubleRow,
                        )
                primt = primt_pool.tile([128, SUP], dt.bfloat16)
                nc.scalar.copy(primt[:], psp[:])
                primt_all[sp] = primt

            def mm2_d(sp, split=False):
                """uh_dmaj + t0 for the 4 chunks of super sp (feeds r0).
                split: do half the psum->sbuf copies on the (idle) Vector
                engine so the ACT copy chain isn't serial."""
                for c in range(CPS):
                    s = sp * CPS + c
                    bi = chunk_to_batch[s]
                    k = s - batches[bi][0]
                    lhsT = primt_all[sp][:, c * 128 : (c + 1) * 128]
                    psu_d = psu_pool.tile(
                        [128, N * D], dt.float32, tag="psu", name="psu_d"
                    )
                    nc.tensor.matmul(
                        psu_d[:], lhsT, capsd_sb[:], start=True, stop=True
                    )
                    pst = pst_pool.tile([128, CAP_DIM], dt.float32)
                    nc.tensor.matmul(
                        pst[:], lhsT, capsum_sb[:], start=True, stop=True
                    )
                    dst = uhd_all[bi][:, k, :, :]
                    src = psu_d.rearrange("p (n d) -> p n d", n=N)
                    if split and c >= 2:
                        nc.vector.tensor_copy(dst, src)
                    else:
                        nc.scalar.copy(dst, src)
                    nc.scalar.copy(t_all[bi][:, k, 0:D], pst[:])

            def mm2_n(sp, split=False):
                """uh_nmaj for the 4 chunks of super sp (feeds r1/r2)."""
                for c in range(CPS):
                    s = sp * CPS + c
                    bi = chunk_to_batch[s]
                    k = s - batches[bi][0]
                    lhsT = primt_all[sp][:, c * 128 : (c + 1) * 128]
                    psu_n = psu_pool.tile(
                        [128, D * N], dt.float32, tag="psu", name="psu_n"
                    )
                    nc.tensor.matmul(
                        psu_n[:], lhsT, capsn_sb[:], start=True, stop=True
                    )
                    dst = uhn_all[bi][:, k, 0:D, :]
                    src = psu_n.rearrange("p (d n) -> p d n", d=D)
                    if split and c >= 2:
                        nc.vector.tensor_copy(dst, src)
                    else:
                        nc.scalar.copy(dst, src)

            # ---- emission schedule: two groups of 8 chunks (2 supers each),
            # ordered by data arrival so no engine FIFO entry blocks a
            # later-emitted but earlier-ready op.
            mm1_super(0)
            mm2_d(0, split=True)
            routing_r0(0, 0, 4, tree_now=True)
            mm2_n(0, split=True)
            mm1_super(1)
            mm2_d(1, split=True)
            routing_r0(0, 4, 6, tree_now=True)
            routing_r0_fin(0, tree_done=True)
            routing_r0(1, 0, 2, tree_now=True)
            mm2_n(1)
            routing_round(0, 1)
            mm1_super(2)
            mm2_d(2)
            routing_r0(1, 2, 6, tree_now=True)
            mm2_n(2)
            routing_round(0, 2)
            mm1_super(3)
            mm2_d(3)
            routing_r0(1, 6, 10, tree_now=True)
            routing_r0_fin(1, tree_done=True)
            mm2_n(3)
            routing_round(1, 1)
            routing_round(1, 2)
            # transpose [128 rows, 16 chunks] -> [16, 128] on PE so the
            # output leaves as ONE dma with 512B/partition descriptors
            # (the naive scattered store cost ~11us of tail)
            ot = pst_pool.tile([16, 128], dt.float32, tag="otr", name="otr")
            nc.tensor.transpose(ot[:], out_sb[:], ident_sb[:])
            nc.scalar.copy(outf_sb[:], ot[:])
            nc.sync.dma_start(out=out_ap[:, :], in_=outf_sb[:])

    nc.compile()
    return nc


def _prep_params(W, b_lin, out_caps, hidden=HIDDEN):
    NH = hidden // 128
    w_f = np.ascontiguousarray(
        (W.astype(np.float32) * W_SCALE)
        .reshape(NH, 128, NUM_CAPS * CAP_DIM)
        .transpose(1, 0, 2)
    ).astype(FP8)
    # full_caps[ic, n, d]: block-diagonal per (o,i): rows i*16..i*16+15
    full_caps = np.zeros((128, N_ROUTE, CAP_DIM), np.float32)
    for o in range(NUM_OBJ):
        for i in range(NUM_CAPS):
            full_caps[
                i * CAP_DIM : (i + 1) * CAP_DIM, o * NUM_CAPS + i, :
            ] = out_caps[o, i]
    full_caps /= W_SCALE
    capsd = np.ascontiguousarray(full_caps.reshape(128, -1)).astype(BF16)
    capsn = np.ascontiguousarray(
        full_caps.transpose(0, 2, 1).reshape(128, -1)
    ).astype(BF16)
    capsum = np.ascontiguousarray(full_caps.sum(1)).astype(BF16)
    bias_row = np.concatenate(
        [
            b_lin.astype(np.float32).reshape(1, 128) * W_SCALE,
            np.ones((1, 128), np.float32),
        ],
        axis=1,
    ).astype(BF16)
    return w_f, capsd, capsn, capsum, bias_row


_NC_CACHE = {}


def kernel(x, W, b_lin, out_caps):
    global LAST_EXEC_TIME_NS
    from concourse.bass_utils import run_bass_kernel_spmd

    x = np.asarray(x)
    W = np.asarray(W)
    b_lin = np.asarray(b_lin)
    out_caps = np.asarray(out_caps)
    bsz, hidden = x.shape
    b_sh = bsz // N_CORES
    NH = hidden // 128
    SUP = 512
    NSUP = b_sh // SUP

    key = (hidden, b_sh)
    if key not in _NC_CACHE:
        _NC_CACHE[key] = build_bass(hidden=hidden, b_sh=b_sh)
    nc = _NC_CACHE[key]

    w_f, capsd, capsn, capsum, bias_row = _prep_params(W, b_lin, out_caps, hidden)

    in_maps = []
    for i in range(N_CORES):
        shard = x[i * b_sh : (i + 1) * b_sh]
        # [sp, pc, p, hq, b]: 4 contiguous 512KB piece-DMAs per super
        # (4KB contiguous per partition per piece)
        NPC = 4
        HQ = NH // NPC
        xt = np.ascontiguousarray(
            shard.reshape(NSUP, SUP, NPC, HQ, 128).transpose(0, 2, 4, 3, 1)
        ).astype(FP8)
        in_maps.append(
            {
                "xt": xt,
                "w": w_f,
                "capsd": capsd,
                "capsn": capsn,
                "capsum": capsum,
                "bias": bias_row,
                "ident": np.eye(128, dtype=np.float32),
            }
        )

    res = run_bass_kernel_spmd(
        nc,
        in_maps,
        core_ids=list(range(N_CORES)),
        trace=bool(int(os.environ.get("BASS_TRACE", "0") or "0")),
    )
    LAST_EXEC_TIME_NS = res.exec_time_ns
    return np.concatenate(
        [res.results[i]["out"].reshape(-1) for i in range(N_CORES)]
    )
